# revision 34
# baseline (speedup 1.0000x reference)
"""3-level db4 wavelet low/high split for (32, 64, 16384) fp32 on 8 TRN2 NeuronCores.

Math: the reference computes wavedec (3-level db4, symmetric padding), then two
waverecs: `low` (details zeroed) and `high` (approximation zeroed).  Wavelets
give perfect reconstruction, so low + high == x and only the lowpass path is
needed: low = G @ (H @ x_row) with H (2054 x 16384) the composite 3-level
lowpass analysis operator (symmetric extension folded in) and G (16384 x 2054)
the lowpass synthesis operator; high = x - low on-chip.

Both operators are banded and 8-periodic, so all their 128-wide tiles dedupe
to 11 (stage 1) + 3 (stage 2) distinct weight tiles.

The whole pipeline runs in fp16 (tolerance is 2e-2 of global max; fp16 lands
~6e-4): halves DMA traffic vs fp32 (24 MB/core, the roofline term) and PE
streaming cost, and enables DVE 16-bit double-rate.

Device pipeline per core (256 rows = 2 row-groups of 128 partitions):
 - PE transpose-mode: x_row [rows, pos] -> x_sig [pos, rows] per 128-block,
   batched 4 pos-blocks per PSUM tile, single copy to SBUF
 - stage 1 (PE, f16): a3_sig[ab] = sum_pb HT_tile(pb,ab).T @ x_sig[pb]
 - stage 2 (PE, f16): low_row = a3_sig-as-stationary @ GT_tile  -> row-major
 - ACT: PSUM->SBUF copies (low, a3); DVE: xsig copies + high = x - low (f16, 2x)
 - stage-2 psum paired into [128,1024] 2-bank tiles (one copy+sub per pair);
   outputs staged in [128, 4096] f16 tiles, one dma_start per tile
Sharding: batch*feature rows 2048 -> 256 rows per core, zero communication.
"""

import numpy as np
import scipy.sparse as sp

import concourse.bacc as bacc
import concourse.tile as tile
from concourse import mybir
from concourse.bass_utils import run_bass_kernel_spmd

F32 = mybir.dt.float32
F16 = mybir.dt.float16

DEC_LO = np.array([-0.010597401785069032, 0.032883011666982945, 0.030841381835986965,
                   -0.18703481171888114, -0.02798376941698385, 0.6308807679295904,
                   0.7148465705525415, 0.23037781330885523], dtype=np.float64)
REC_LO = DEC_LO[::-1].copy()
F = 8
N_CORES = 8


def _symidx(n):
    idx = np.concatenate([np.arange(6, -1, -1), np.arange(n), np.arange(n - 1, n - 8, -1)])
    return idx[1:]


def _dwt_lo_mat(n):
    ext_idx = _symidx(n)
    lout = (n + 13 - F) // 2 + 1
    filt = DEC_LO[::-1]
    rows = np.repeat(np.arange(lout), F)
    cols = ext_idx[(2 * np.arange(lout)[:, None] + np.arange(F)[None, :]).ravel()]
    vals = np.tile(filt, lout)
    return sp.coo_matrix((vals, (rows, cols)), shape=(lout, n)).tocsr()


def _idwt_lo_mat(n):
    lout = 2 * n + 1 - F + 1
    filt = REC_LO[::-1]
    rows, cols, vals = [], [], []
    i = np.arange(lout)
    for k in range(F):
        pos = i + k
        m = (pos % 2 == 1)
        rows.append(i[m])
        cols.append((pos[m] - 1) // 2)
        vals.append(np.full(int(m.sum()), filt[k]))
    return sp.coo_matrix(
        (np.concatenate(vals), (np.concatenate(rows), np.concatenate(cols))),
        shape=(lout, n)).tocsr()


def _build_H_G(L, level=3):
    H = sp.identity(L, format="csr")
    lens = []
    n = L
    for _ in range(level):
        lens.append(n)
        D = _dwt_lo_mat(n)
        H = D @ H
        n = D.shape[0]
    G = sp.identity(n, format="csr")
    a_len = n
    for ln in lens[::-1]:
        d_len = (ln + F - 1) // 2
        if a_len == d_len + 1:
            G = sp.identity(a_len, format="csr")[:-1] @ G
            a_len -= 1
        U = _idwt_lo_mat(a_len)
        G = U @ G
        a_len = U.shape[0]
    return H, G


def _build_plan(L):
    H, G = _build_H_G(L)
    na = H.shape[0]
    nab = (na + 127) // 128
    HTp = np.zeros((L, nab * 128), np.float16)
    HTp[:, :na] = np.asarray(H.T.todense(), np.float16)
    GTp = np.zeros((nab * 128, L), np.float16)
    GTp[:na, :] = np.asarray(G.T.todense(), np.float16)

    npb = L // 128
    nw = L // 512

    s1_tiles, s1map = {}, [[] for _ in range(nab)]
    for ab in range(nab):
        for pb in range(npb):
            t = HTp[128 * pb:128 * pb + 128, 128 * ab:128 * ab + 128]
            if np.any(t):
                tid = s1_tiles.setdefault(t.tobytes(), len(s1_tiles))
                s1map[ab].append((pb, tid))
    w1 = np.zeros((128, 128 * len(s1_tiles)), np.float16)
    for key, tid in s1_tiles.items():
        w1[:, 128 * tid:128 * tid + 128] = np.frombuffer(key, np.float16).reshape(128, 128)

    s2_tiles, s2map = {}, [[] for _ in range(nw)]
    for ab in range(nab):
        for w in range(nw):
            t = GTp[128 * ab:128 * ab + 128, 512 * w:512 * w + 512]
            if np.any(t):
                nzc = np.nonzero(np.any(t != 0, axis=0))[0]
                tid = s2_tiles.setdefault(t.tobytes(), len(s2_tiles))
                s2map[w].append((ab, tid, int(nzc.min()), int(nzc.max()) + 1))
    for w in range(nw):
        s2map[w].sort(key=lambda e: -(e[3] - e[2]))  # widest (full-bank) first
    w2 = np.zeros((128, 512 * len(s2_tiles)), np.float16)
    for key, tid in s2_tiles.items():
        w2[:, 512 * tid:512 * tid + 512] = np.frombuffer(key, np.float16).reshape(128, 512)

    first_need = {}
    for ab in range(nab):
        for pb, _ in s1map[ab]:
            first_need.setdefault(pb, ab)
    tsched = [[] for _ in range(nab)]
    for pb, ab in first_need.items():
        tsched[ab].append(pb)
    wsched = [[] for _ in range(nab)]
    for w in range(nw):
        wsched[max(ab for ab, _, _, _ in s2map[w])].append(w)
    for lst in tsched:
        lst.sort()
    for lst in wsched:
        lst.sort()

    return dict(L=L, nab=nab, npb=npb, nw=nw, w1=w1, w2=w2,
                s1map=s1map, s2map=s2map, tsched=tsched, wsched=wsched)


def _build_program(plan, rows, chunk=4096, xsig_bufs=10, xchunk_bufs=5,
                   out_group=8, tr_group=4):
    L, nab, nw = plan["L"], plan["nab"], plan["nw"]
    nrg = rows // 128
    assert rows % 128 == 0 and nrg == 2
    nch = L // chunk
    pb_per_chunk = chunk // 128
    assert nw % out_group == 0 and out_group % 2 == 0

    nc = bacc.Bacc("TRN2", target_bir_lowering=False, debug=False)
    x_d = nc.dram_tensor("x", [rows, L], F16, kind="ExternalInput").ap()
    w1_d = nc.dram_tensor("w1", list(plan["w1"].shape), F16, kind="ExternalInput").ap()
    w2_d = nc.dram_tensor("w2", list(plan["w2"].shape), F16, kind="ExternalInput").ap()
    id_d = nc.dram_tensor("ident", [128, 128], F16, kind="ExternalInput").ap()
    low_d = nc.dram_tensor("low", [rows, L], F16, kind="ExternalOutput").ap()
    high_d = nc.dram_tensor("high", [rows, L], F16, kind="ExternalOutput").ap()

    with tile.TileContext(nc) as tc:
        with tc.tile_pool(name="sbw", bufs=1) as sbw, \
             tc.tile_pool(name="sbx", bufs=xchunk_bufs) as sbx, \
             tc.tile_pool(name="sbxs", bufs=xsig_bufs) as sbxs, \
             tc.tile_pool(name="sba3", bufs=nab) as sba3, \
             tc.tile_pool(name="sbo", bufs=2) as sbo, \
             tc.tile_pool(name="pst", bufs=2, space="PSUM") as pst, \
             tc.tile_pool(name="psa", bufs=2, space="PSUM") as psa, \
             tc.tile_pool(name="ps2", bufs=1, space="PSUM") as ps2p:

            xch, xsig, a3 = {}, {}, {}
            chunks_issued = set()
            lo_st = [None] * nrg
            hi_st = [None] * nrg
            po_pair = [None] * nrg
            w_seen = [[] for _ in range(nrg)]

            def ensure_chunk(c):
                if c in chunks_issued or c >= nch:
                    return
                chunks_issued.add(c)
                # first chunk split into slices (smallest first, on the faster
                # HWDGE sync queue) so the first transposes start early
                bounds = [0, 512, 1024, 2048, 3072, chunk] if c == 0 else [0, chunk]
                for rg in range(nrg):
                    xt = sbx.tile([128, chunk], F16, tag=f"x{rg}")
                    for i in range(len(bounds) - 1):
                        eng = nc.sync if (c == 0 and i == 0) else nc.gpsimd
                        eng.dma_start(
                            xt[:, bounds[i]:bounds[i + 1]],
                            x_d[rg * 128:(rg + 1) * 128,
                                c * chunk + bounds[i]:c * chunk + bounds[i + 1]])
                    xch[(rg, c)] = xt

            ensure_chunk(0)
            idt = sbw.tile([128, 128], F16, tag="idt")
            nc.sync.dma_start(idt[:], id_d[:])
            w1t = sbw.tile(list(plan["w1"].shape), F16, tag="w1t")
            wq = plan["w1"].shape[1] // 2
            for i in range(2):
                nc.sync.dma_start(w1t[:, i * wq:(i + 1) * wq], w1_d[:, i * wq:(i + 1) * wq])
            w2t = sbw.tile(list(plan["w2"].shape), F16, tag="w2t")
            wq2 = plan["w2"].shape[1] // 2
            for i in range(2):
                nc.sync.dma_start(w2t[:, i * wq2:(i + 1) * wq2], w2_d[:, i * wq2:(i + 1) * wq2])

            def do_stage2(k):
                last_g = nw // out_group - 1
                for w in plan["wsched"][k]:
                    g, half = w // out_group, w % 2
                    for rg in range(nrg):
                        w_seen[rg].append(w)
                        if w % out_group == 0:
                            lo_st[rg] = sbo.tile([128, 512 * out_group], F16,
                                                 tag=f"lo{rg}", name=f"lo{rg}")
                            hi_st[rg] = sbo.tile([128, 512 * out_group], F16,
                                                 tag=f"hi{rg}", name=f"hi{rg}")
                        if half == 0:
                            po_pair[rg] = ps2p.tile([128, 1024], F32,
                                                    tag=f"s2r{rg}", name=f"po{rg}")
                        po = po_pair[rg]
                        base = half * 512
                        ents2 = plan["s2map"][w]
                        for j, (ab, tid, clo, chi) in enumerate(ents2):
                            nc.tensor.matmul(
                                po[:, base + clo:base + chi],
                                a3[ab][:, rg * 128:(rg + 1) * 128],
                                w2t[:, 512 * tid + clo:512 * tid + chi],
                                start=(j == 0), stop=(j == len(ents2) - 1))
                        if half != 1:
                            continue
                        sl = ((w - 1) % out_group) * 512
                        c = (512 * (w - 1)) // chunk
                        off = (512 * (w - 1)) % chunk
                        r0, r1 = rg * 128, (rg + 1) * 128
                        q = w % out_group
                        nc.scalar.copy(lo_st[rg][:, sl:sl + 1024], po[:])
                        if g == last_g:
                            # taper: drain the last group in 2048-col pieces to
                            # shrink the tail (tail is issue-rate bound, so not
                            # per-pair: 8 taper calls total, not 16)
                            if q % 4 == 3:
                                p0 = (q - 3) * 512
                                w0 = 512 * out_group * g + p0
                                nc.sync.dma_start(low_d[r0:r1, w0:w0 + 2048],
                                                  lo_st[rg][:, p0:p0 + 2048])
                        elif q == out_group - 1:
                            w0 = 512 * out_group * g
                            nc.sync.dma_start(
                                low_d[r0:r1, w0:w0 + 512 * out_group], lo_st[rg][:])
                        nc.vector.tensor_sub(
                            hi_st[rg][:, sl:sl + 1024],
                            xch[(rg, c)][:, off:off + 1024],
                            lo_st[rg][:, sl:sl + 1024])
                        if g == last_g:
                            if q % 4 == 3:
                                p0 = (q - 3) * 512
                                w0 = 512 * out_group * g + p0
                                nc.sync.dma_start(high_d[r0:r1, w0:w0 + 2048],
                                                  hi_st[rg][:, p0:p0 + 2048])
                        elif q == out_group - 1:
                            w0 = 512 * out_group * g
                            nc.sync.dma_start(
                                high_d[r0:r1, w0:w0 + 512 * out_group], hi_st[rg][:])

            def do_transposes(k):
                pbs = plan["tsched"][k]
                for g0 in range(0, len(pbs), tr_group):
                    grp = pbs[g0:g0 + tr_group]
                    for pb in grp:
                        c = pb // pb_per_chunk
                        ensure_chunk(c)
                        ensure_chunk(c + 1)
                    pt = pst.tile([128, 256 * tr_group], F16, tag="pt")
                    for j, pb in enumerate(grp):
                        c = pb // pb_per_chunk
                        off = (pb % pb_per_chunk) * 128
                        for rg in range(nrg):
                            nc.tensor.transpose(
                                pt[:, j * 256 + rg * 128:j * 256 + (rg + 1) * 128],
                                xch[(rg, c)][:, off:off + 128], idt[:])
                    xs = sbxs.tile([128, 256 * tr_group], F16, tag="xs")
                    span = 256 * len(grp)
                    nc.vector.tensor_copy(xs[:, :span], pt[:, :span])
                    for j, pb in enumerate(grp):
                        xsig[pb] = (xs, j)

            # transposes run one ab AHEAD (their xsig copies get a full
            # ab-cycle), stage 2 one ab BEHIND (its a3 copy + psum pair
            # recycling likewise), so the in-order PE queue never stalls
            do_transposes(0)
            for k in range(nab):
                if k + 1 < nab:
                    do_transposes(k + 1)
                if k > 0:
                    do_stage2(k - 1)

                pa = psa.tile([128, 128 * nrg], F32, tag="pa")
                ents = plan["s1map"][k]
                for i, (pb, tid) in enumerate(ents):
                    xs, j = xsig[pb]
                    nc.tensor.matmul(
                        pa[:], w1t[:, 128 * tid:128 * tid + 128],
                        xs[:, j * 256:(j + 1) * 256],
                        start=(i == 0), stop=(i == len(ents) - 1))
                a3t = sba3.tile([128, 128 * nrg], F16, tag="a3")
                nc.scalar.copy(a3t[:], pa[:])
                a3[k] = a3t

            do_stage2(nab - 1)

            for rg in range(nrg):
                assert w_seen[rg] == list(range(nw)), w_seen[rg]

    nc.compile()
    return nc


_CACHE = {}


def _get_compiled(rows, L):
    key = (rows, L)
    if key not in _CACHE:
        plan = _build_plan(L)
        nc = _build_program(plan, rows=rows)
        _CACHE[key] = (plan, nc)
    return _CACHE[key]


def make_in_maps(x2f16, plan, rows):
    ident = np.eye(128, dtype=np.float16)
    return [{
        "x": np.ascontiguousarray(x2f16[c * rows:(c + 1) * rows]),
        "w1": plan["w1"], "w2": plan["w2"], "ident": ident,
    } for c in range(N_CORES)]


def kernel(x):
    x = np.asarray(x)
    B, Fd, L = x.shape
    in_dtype = x.dtype
    xf = x.reshape(B * Fd, L).astype(np.float16)
    rows = (B * Fd) // N_CORES

    plan, nc = _get_compiled(rows, L)
    in_maps = make_in_maps(xf, plan, rows)

    res = run_bass_kernel_spmd(nc, in_maps, list(range(N_CORES)))
    low = np.concatenate([r["low"] for r in res.results], axis=0).reshape(B, Fd, L)
    high = np.concatenate([r["high"] for r in res.results], axis=0).reshape(B, Fd, L)
    return low.astype(in_dtype, copy=False), high.astype(in_dtype, copy=False)


# revision 35
# speedup vs baseline: 1.0188x; 1.0188x over previous
"""3-level db4 wavelet low/high split for (32, 64, 16384) fp32 on 8 TRN2 NeuronCores.

Math: the reference computes wavedec (3-level db4, symmetric padding), then two
waverecs: `low` (details zeroed) and `high` (approximation zeroed).  Wavelets
give perfect reconstruction, so low + high == x and only the lowpass path is
needed: low = G @ (H @ x_row) with H (2054 x 16384) the composite 3-level
lowpass analysis operator (symmetric extension folded in) and G (16384 x 2054)
the lowpass synthesis operator; high = x - low on-chip.

Both operators are banded and 8-periodic, so all their 128-wide tiles dedupe
to 11 (stage 1) + 3 (stage 2) distinct weight tiles.

The whole pipeline runs in fp16 (tolerance is 2e-2 of global max; fp16 lands
~6e-4): halves DMA traffic vs fp32 (24 MB/core, the roofline term) and PE
streaming cost, and enables DVE 16-bit double-rate.

Device pipeline per core (256 rows = 2 row-groups of 128 partitions):
 - PE transpose-mode: x_row [rows, pos] -> x_sig [pos, rows] per 128-block,
   batched 4 pos-blocks per PSUM tile, single copy to SBUF
 - stage 1 (PE, f16): a3_sig[ab] = sum_pb HT_tile(pb,ab).T @ x_sig[pb]
 - stage 2 (PE, f16): low_row = a3_sig-as-stationary @ GT_tile  -> row-major
 - ACT: PSUM->SBUF copies (low, a3); DVE: xsig copies + high = x - low (f16, 2x)
 - stage-2 psum paired into [128,1024] 2-bank tiles (one copy+sub per pair);
   outputs staged in [128, 4096] f16 tiles, one dma_start per tile
Sharding: batch*feature rows 2048 -> 256 rows per core, zero communication.
"""

import numpy as np
import scipy.sparse as sp

import concourse.bacc as bacc
import concourse.tile as tile
from concourse import mybir
from concourse.bass_utils import run_bass_kernel_spmd

F32 = mybir.dt.float32
F16 = mybir.dt.float16

DEC_LO = np.array([-0.010597401785069032, 0.032883011666982945, 0.030841381835986965,
                   -0.18703481171888114, -0.02798376941698385, 0.6308807679295904,
                   0.7148465705525415, 0.23037781330885523], dtype=np.float64)
REC_LO = DEC_LO[::-1].copy()
F = 8
N_CORES = 8


def _symidx(n):
    idx = np.concatenate([np.arange(6, -1, -1), np.arange(n), np.arange(n - 1, n - 8, -1)])
    return idx[1:]


def _dwt_lo_mat(n):
    ext_idx = _symidx(n)
    lout = (n + 13 - F) // 2 + 1
    filt = DEC_LO[::-1]
    rows = np.repeat(np.arange(lout), F)
    cols = ext_idx[(2 * np.arange(lout)[:, None] + np.arange(F)[None, :]).ravel()]
    vals = np.tile(filt, lout)
    return sp.coo_matrix((vals, (rows, cols)), shape=(lout, n)).tocsr()


def _idwt_lo_mat(n):
    lout = 2 * n + 1 - F + 1
    filt = REC_LO[::-1]
    rows, cols, vals = [], [], []
    i = np.arange(lout)
    for k in range(F):
        pos = i + k
        m = (pos % 2 == 1)
        rows.append(i[m])
        cols.append((pos[m] - 1) // 2)
        vals.append(np.full(int(m.sum()), filt[k]))
    return sp.coo_matrix(
        (np.concatenate(vals), (np.concatenate(rows), np.concatenate(cols))),
        shape=(lout, n)).tocsr()


def _build_H_G(L, level=3):
    H = sp.identity(L, format="csr")
    lens = []
    n = L
    for _ in range(level):
        lens.append(n)
        D = _dwt_lo_mat(n)
        H = D @ H
        n = D.shape[0]
    G = sp.identity(n, format="csr")
    a_len = n
    for ln in lens[::-1]:
        d_len = (ln + F - 1) // 2
        if a_len == d_len + 1:
            G = sp.identity(a_len, format="csr")[:-1] @ G
            a_len -= 1
        U = _idwt_lo_mat(a_len)
        G = U @ G
        a_len = U.shape[0]
    return H, G


def _build_plan(L):
    H, G = _build_H_G(L)
    na = H.shape[0]
    nab = (na + 127) // 128
    HTp = np.zeros((L, nab * 128), np.float16)
    HTp[:, :na] = np.asarray(H.T.todense(), np.float16)
    GTp = np.zeros((nab * 128, L), np.float16)
    GTp[:na, :] = np.asarray(G.T.todense(), np.float16)

    npb = L // 128
    nw = L // 512

    s1_tiles, s1map = {}, [[] for _ in range(nab)]
    for ab in range(nab):
        for pb in range(npb):
            t = HTp[128 * pb:128 * pb + 128, 128 * ab:128 * ab + 128]
            if np.any(t):
                tid = s1_tiles.setdefault(t.tobytes(), len(s1_tiles))
                s1map[ab].append((pb, tid))
    w1 = np.zeros((128, 128 * len(s1_tiles)), np.float16)
    for key, tid in s1_tiles.items():
        w1[:, 128 * tid:128 * tid + 128] = np.frombuffer(key, np.float16).reshape(128, 128)

    s2_tiles, s2map = {}, [[] for _ in range(nw)]
    for ab in range(nab):
        for w in range(nw):
            t = GTp[128 * ab:128 * ab + 128, 512 * w:512 * w + 512]
            if np.any(t):
                nzc = np.nonzero(np.any(t != 0, axis=0))[0]
                tid = s2_tiles.setdefault(t.tobytes(), len(s2_tiles))
                s2map[w].append((ab, tid, int(nzc.min()), int(nzc.max()) + 1))
    for w in range(nw):
        s2map[w].sort(key=lambda e: -(e[3] - e[2]))  # widest (full-bank) first
    w2 = np.zeros((128, 512 * len(s2_tiles)), np.float16)
    for key, tid in s2_tiles.items():
        w2[:, 512 * tid:512 * tid + 512] = np.frombuffer(key, np.float16).reshape(128, 512)

    first_need = {}
    for ab in range(nab):
        for pb, _ in s1map[ab]:
            first_need.setdefault(pb, ab)
    tsched = [[] for _ in range(nab)]
    for pb, ab in first_need.items():
        tsched[ab].append(pb)
    wsched = [[] for _ in range(nab)]
    for w in range(nw):
        wsched[max(ab for ab, _, _, _ in s2map[w])].append(w)
    for lst in tsched:
        lst.sort()
    for lst in wsched:
        lst.sort()

    return dict(L=L, nab=nab, npb=npb, nw=nw, w1=w1, w2=w2,
                s1map=s1map, s2map=s2map, tsched=tsched, wsched=wsched)


def _build_program(plan, rows, chunk=4096, xsig_bufs=10, xchunk_bufs=5,
                   out_group=8, tr_group=4):
    L, nab, nw = plan["L"], plan["nab"], plan["nw"]
    nrg = rows // 128
    assert rows % 128 == 0 and nrg == 2
    nch = L // chunk
    pb_per_chunk = chunk // 128
    assert nw % out_group == 0 and out_group % 2 == 0

    nc = bacc.Bacc("TRN2", target_bir_lowering=False, debug=False)
    x_d = nc.dram_tensor("x", [rows, L], F16, kind="ExternalInput").ap()
    w1_d = nc.dram_tensor("w1", list(plan["w1"].shape), F16, kind="ExternalInput").ap()
    w2_d = nc.dram_tensor("w2", list(plan["w2"].shape), F16, kind="ExternalInput").ap()
    id_d = nc.dram_tensor("ident", [128, 128], F16, kind="ExternalInput").ap()
    low_d = nc.dram_tensor("low", [rows, L], F16, kind="ExternalOutput").ap()
    high_d = nc.dram_tensor("high", [rows, L], F16, kind="ExternalOutput").ap()

    with tile.TileContext(nc) as tc:
        with tc.tile_pool(name="sbw", bufs=1) as sbw, \
             tc.tile_pool(name="sbx", bufs=xchunk_bufs) as sbx, \
             tc.tile_pool(name="sbxs", bufs=xsig_bufs) as sbxs, \
             tc.tile_pool(name="sba3", bufs=nab) as sba3, \
             tc.tile_pool(name="sbo", bufs=2) as sbo, \
             tc.tile_pool(name="pst", bufs=2, space="PSUM") as pst, \
             tc.tile_pool(name="psa", bufs=2, space="PSUM") as psa, \
             tc.tile_pool(name="ps2", bufs=1, space="PSUM") as ps2p:

            xch, xsig, a3 = {}, {}, {}
            chunks_issued = set()
            lo_st = [None] * nrg
            hi_st = [None] * nrg
            po_pair = [None] * nrg
            w_seen = [[] for _ in range(nrg)]

            def ensure_chunk(c):
                if c in chunks_issued or c >= nch:
                    return
                chunks_issued.add(c)
                # first chunk split into slices (smallest first, on the faster
                # HWDGE sync queue) so the first transposes start early
                bounds = [0, 512, 1024, 2048, 3072, chunk] if c == 0 else [0, chunk]
                for rg in range(nrg):
                    xt = sbx.tile([128, chunk], F16, tag=f"x{rg}")
                    for i in range(len(bounds) - 1):
                        eng = nc.sync if (c == 0 and i == 0) else nc.gpsimd
                        eng.dma_start(
                            xt[:, bounds[i]:bounds[i + 1]],
                            x_d[rg * 128:(rg + 1) * 128,
                                c * chunk + bounds[i]:c * chunk + bounds[i + 1]])
                    xch[(rg, c)] = xt

            ensure_chunk(0)
            idt = sbw.tile([128, 128], F16, tag="idt")
            # scalar HWDGE ring: lands in parallel with the first x slices on
            # the sync ring, so the first transpose waits on neither
            nc.scalar.dma_start(idt[:], id_d[:])
            w1t = sbw.tile(list(plan["w1"].shape), F16, tag="w1t")
            wq = plan["w1"].shape[1] // 2
            for i in range(2):
                nc.sync.dma_start(w1t[:, i * wq:(i + 1) * wq], w1_d[:, i * wq:(i + 1) * wq])
            w2t = sbw.tile(list(plan["w2"].shape), F16, tag="w2t")
            wq2 = plan["w2"].shape[1] // 2
            for i in range(2):
                nc.sync.dma_start(w2t[:, i * wq2:(i + 1) * wq2], w2_d[:, i * wq2:(i + 1) * wq2])

            def do_stage2(k):
                last_g = nw // out_group - 1
                for w in plan["wsched"][k]:
                    g, half = w // out_group, w % 2
                    for rg in range(nrg):
                        w_seen[rg].append(w)
                        if w % out_group == 0:
                            lo_st[rg] = sbo.tile([128, 512 * out_group], F16,
                                                 tag=f"lo{rg}", name=f"lo{rg}")
                            hi_st[rg] = sbo.tile([128, 512 * out_group], F16,
                                                 tag=f"hi{rg}", name=f"hi{rg}")
                        if half == 0:
                            po_pair[rg] = ps2p.tile([128, 1024], F32,
                                                    tag=f"s2r{rg}", name=f"po{rg}")
                        po = po_pair[rg]
                        base = half * 512
                        ents2 = plan["s2map"][w]
                        for j, (ab, tid, clo, chi) in enumerate(ents2):
                            nc.tensor.matmul(
                                po[:, base + clo:base + chi],
                                a3[ab][:, rg * 128:(rg + 1) * 128],
                                w2t[:, 512 * tid + clo:512 * tid + chi],
                                start=(j == 0), stop=(j == len(ents2) - 1))
                        if half != 1:
                            continue
                        sl = ((w - 1) % out_group) * 512
                        c = (512 * (w - 1)) // chunk
                        off = (512 * (w - 1)) % chunk
                        r0, r1 = rg * 128, (rg + 1) * 128
                        q = w % out_group
                        nc.scalar.copy(lo_st[rg][:, sl:sl + 1024], po[:])
                        if g == last_g:
                            # taper: drain the last group in 2048-col pieces to
                            # shrink the tail (tail is issue-rate bound, so not
                            # per-pair: 8 taper calls total, not 16)
                            if q % 4 == 3:
                                p0 = (q - 3) * 512
                                w0 = 512 * out_group * g + p0
                                nc.sync.dma_start(low_d[r0:r1, w0:w0 + 2048],
                                                  lo_st[rg][:, p0:p0 + 2048])
                        elif q == out_group - 1:
                            w0 = 512 * out_group * g
                            nc.sync.dma_start(
                                low_d[r0:r1, w0:w0 + 512 * out_group], lo_st[rg][:])
                        nc.vector.tensor_sub(
                            hi_st[rg][:, sl:sl + 1024],
                            xch[(rg, c)][:, off:off + 1024],
                            lo_st[rg][:, sl:sl + 1024])
                        if g == last_g:
                            if q % 4 == 3:
                                p0 = (q - 3) * 512
                                w0 = 512 * out_group * g + p0
                                nc.sync.dma_start(high_d[r0:r1, w0:w0 + 2048],
                                                  hi_st[rg][:, p0:p0 + 2048])
                        elif q == out_group - 1:
                            w0 = 512 * out_group * g
                            nc.sync.dma_start(
                                high_d[r0:r1, w0:w0 + 512 * out_group], hi_st[rg][:])

            def do_transposes(k):
                pbs = plan["tsched"][k]
                for g0 in range(0, len(pbs), tr_group):
                    grp = pbs[g0:g0 + tr_group]
                    for pb in grp:
                        c = pb // pb_per_chunk
                        ensure_chunk(c)
                        ensure_chunk(c + 1)
                    pt = pst.tile([128, 256 * tr_group], F16, tag="pt")
                    for j, pb in enumerate(grp):
                        c = pb // pb_per_chunk
                        off = (pb % pb_per_chunk) * 128
                        for rg in range(nrg):
                            nc.tensor.transpose(
                                pt[:, j * 256 + rg * 128:j * 256 + (rg + 1) * 128],
                                xch[(rg, c)][:, off:off + 128], idt[:])
                    xs = sbxs.tile([128, 256 * tr_group], F16, tag="xs")
                    span = 256 * len(grp)
                    nc.vector.tensor_copy(xs[:, :span], pt[:, :span])
                    for j, pb in enumerate(grp):
                        xsig[pb] = (xs, j)

            # transposes run one ab AHEAD (their xsig copies get a full
            # ab-cycle), stage 2 one ab BEHIND (its a3 copy + psum pair
            # recycling likewise), so the in-order PE queue never stalls
            do_transposes(0)
            for k in range(nab):
                if k + 1 < nab:
                    do_transposes(k + 1)
                if k > 0:
                    do_stage2(k - 1)

                pa = psa.tile([128, 128 * nrg], F32, tag="pa")
                ents = plan["s1map"][k]
                for i, (pb, tid) in enumerate(ents):
                    xs, j = xsig[pb]
                    nc.tensor.matmul(
                        pa[:], w1t[:, 128 * tid:128 * tid + 128],
                        xs[:, j * 256:(j + 1) * 256],
                        start=(i == 0), stop=(i == len(ents) - 1))
                a3t = sba3.tile([128, 128 * nrg], F16, tag="a3")
                nc.scalar.copy(a3t[:], pa[:])
                a3[k] = a3t

            do_stage2(nab - 1)

            for rg in range(nrg):
                assert w_seen[rg] == list(range(nw)), w_seen[rg]

    nc.compile()
    return nc


_CACHE = {}


def _get_compiled(rows, L):
    key = (rows, L)
    if key not in _CACHE:
        plan = _build_plan(L)
        nc = _build_program(plan, rows=rows)
        _CACHE[key] = (plan, nc)
    return _CACHE[key]


def make_in_maps(x2f16, plan, rows):
    ident = np.eye(128, dtype=np.float16)
    return [{
        "x": np.ascontiguousarray(x2f16[c * rows:(c + 1) * rows]),
        "w1": plan["w1"], "w2": plan["w2"], "ident": ident,
    } for c in range(N_CORES)]


def kernel(x):
    x = np.asarray(x)
    B, Fd, L = x.shape
    in_dtype = x.dtype
    xf = x.reshape(B * Fd, L).astype(np.float16)
    rows = (B * Fd) // N_CORES

    plan, nc = _get_compiled(rows, L)
    in_maps = make_in_maps(xf, plan, rows)

    res = run_bass_kernel_spmd(nc, in_maps, list(range(N_CORES)))
    low = np.concatenate([r["low"] for r in res.results], axis=0).reshape(B, Fd, L)
    high = np.concatenate([r["high"] for r in res.results], axis=0).reshape(B, Fd, L)
    return low.astype(in_dtype, copy=False), high.astype(in_dtype, copy=False)
